# revision 26
# baseline (speedup 1.0000x reference)
"""Trainium2 Bass kernel for nn_L2MLoRAqkv (MoE-routed LoRA QKV projection).

Math (per batch b, expert i = idx[b,0]):
    qkv = x @ W.T + bias
    qkv[:, :D]  += (x @ A_q[i]) @ B_q[i] * SCALE
    qkv[:, -D:] += (x @ A_v[i]) @ B_v[i] * SCALE

Strategy: data-parallel over the batch dim (1 batch per NeuronCore, 8 cores).
On the host we gather each batch's expert and fold the rank-8 LoRA update
into the (transposed) projection weight in float64, so the device kernel is
a single dense GEMM per core:
    Y[4096, 3072] = X[4096, 1024] @ W_eff[1024, 3072] + bias

PE floor is 1536 matmuls x 216ns = 331.8us.  Design notes (from traces):
 - LDWEIGHTS issue is only hidden when >=2 consecutive matmuls share the
   stationary operand (a width-1 schedule costs +43ns/mm).  So: phase A
   covers n0+n1 for every token tile (width-2 reuse of each x tile) while
   x streams in chunk by chunk; phase B covers n2..n5 (width-4) with
   everything resident.  Phase A's new-byte demand is ~75GB/s, phase B's
   is zero -- no DMA stall, so the HAM clock gate never re-throttles.
 - Each dma_start costs ~690ns of HWDGE ring trigger time regardless of
   size, so bulk data moves as 1MB single DMAs (8KB contiguous partition
   lines).  Only the startup-critical head data (w n0/n1 + x chunk 0,
   consumed k-step by k-step before the rings are warm) is split into
   per-k 128KB pieces, ring-alternated in exact consumption order.
 - The HAM clock gate (1.2GHz cold) flips to 2.4GHz only after one fully
   busy free-running 3413ns window, and re-throttles after any idle
   window.  gpsimd memsets the ones tile as soon as its queue is up
   (~6.3us); ~12 K=1 warm-ups plus the 6 bias-replication matmuls keep
   the PE gapless until the first head pieces land (~10us).
 - bias ships as one 12KB row and is replicated across partitions on-chip
   via ones[1,128].T @ bias_row matmuls into 2 PSUM banks during warm-up;
   the other groups rotate freely through all 8 banks (phase A head uses
   8, steady-state super-groups 4 -> fully double-buffered, no gaps).
 - Stores coalesce per token tile ([128,1024] in phase A, [128,2048] in
   phase B; 2-4KB lines) and are emitted after all loads so ring FIFO
   order can never block a load on an unfinished drain.  The final block
   runs n-major so its drains/stores overlap the remaining matmuls, and
   the very last 512-col group drains and stores in two halves.
"""

import os
import sys

import numpy as np

for _p in ("/opt/trn_rl_repo",):
    if _p not in sys.path and os.path.isdir(_p):
        sys.path.insert(0, _p)

B = 8          # batches == cores
T = 4096       # tokens per batch
D = 1024       # model dim (contraction K)
N3 = 3072      # qkv output dim
P = 128        # SBUF partitions
NT = 512       # n-tile (one fp32 PSUM bank)
KT = D // P        # 8 k-tiles
NN = N3 // NT      # 6 n-blocks
CH = T // NT       # 8 token chunks
JJ = NT // P       # 4 token tiles per chunk
NA = 2             # n-blocks done in phase A (width-2 lhsT reuse)
NB = NN - NA       # n-blocks done in phase B (width-4 lhsT reuse)
WARM = 48          # ones warm-ups bridging queue-up -> first head pieces
SCALE = 8.0 / 8.0

_NC_CACHE = {}


def _build():
    import concourse.tile as tile
    from concourse import bacc, mybir

    bf16 = mybir.dt.bfloat16
    f32 = mybir.dt.float32

    nc = bacc.Bacc(
        "TRN2",
        target_bir_lowering=False,
        debug=False,
        enable_asserts=False,
        num_devices=B,
    )
    # hb rows: k*P + p, cols: [w(0,k) | x(0,k) | w(1,k)] -- one contiguous
    # 384KB piece per k-step of the head super-group (single trigger+lane).
    hb = nc.dram_tensor("hb", [KT * P, 3 * NT], bf16, kind="ExternalInput").ap()
    # wq rows: (n-NA)*P + p, cols: k*NT + c (one 1MB contiguous slice per n)
    wq = nc.dram_tensor("wq", [NB * P, KT * NT], bf16, kind="ExternalInput").ap()
    # xq rows: (c-1)*P + p, cols: k*NT + cc (one 1MB contiguous chunk per c)
    xq = nc.dram_tensor("xq", [(CH - 1) * P, KT * NT], bf16,
                        kind="ExternalInput").ap()
    biasr = nc.dram_tensor("biasr", [1, N3], f32, kind="ExternalInput").ap()
    y = nc.dram_tensor("y", [T, N3], bf16, kind="ExternalOutput").ap()

    with tile.TileContext(nc) as tc:
        with tc.tile_pool(name="const", bufs=1) as const_pool, \
             tc.tile_pool(name="outp", bufs=1) as out_pool, \
             tc.tile_pool(name="ps", bufs=1, space="PSUM") as psum_pool:

            # head_sb col k*4*NT: [w(0,k) | x(0,k) | w(1,k) | pad]
            # w_sb col ((n-NA)*KT+k)*NT: w[k, n*NT:(n+1)*NT] for n>=NA
            # x_sb col ((c-1)*KT+k)*NT + j*P: xT[k, (c*JJ+j)*P ...] for c>=1
            # NOTE: partition strides are kept powers of two -- a non-pow2
            # stride on the lhsT tile puts LDWEIGHTS on a slow path that
            # is not hidden behind the matmul stream (+43ns per matmul).
            head_sb = const_pool.tile([P, KT * 4 * NT], bf16)
            w_sb = const_pool.tile([P, NB * KT * NT], bf16)
            x_sb = const_pool.tile([P, CH * KT * NT], bf16)
            bias_sb = const_pool.tile([P, N3], f32)
            bias_row = const_pool.tile([1, N3], f32)
            ones_sb = const_pool.tile([1, P], bf16)

            # ones for warm-ups / bias replication: gpsimd's queue is up
            # earliest and is otherwise idle.
            nc.gpsimd.memset(ones_sb[:], 1.0)

            # ---- loads, in exact consumption order, alternating rings ----
            ring = [nc.scalar, nc.sync]
            rn = [0]

            def pick():
                eng = ring[rn[0] % 2]
                rn[0] += 1
                return eng

            nc.scalar.dma_start(bias_row[:], biasr[:])
            # head blob: one contiguous [w(0,k)|x(0,k)|w(1,k)] piece per k,
            # ring-alternated in exactly head k-step consumption order.
            # All of k0 goes first on the sync ring in three sub-pieces (the
            # scalar ring's first transfers start several us later), sized
            # so each lands just before its matmuls: the first k-step runs
            # n-outer so w(1,0) is only needed 4 matmuls in.
            nc.sync.dma_start(head_sb[:, 0 : NT + 2 * P],
                              hb[0:P, 0 : NT + 2 * P])
            nc.sync.dma_start(head_sb[:, NT + 2 * P : 2 * NT],
                              hb[0:P, NT + 2 * P : 2 * NT])
            nc.sync.dma_start(head_sb[:, 2 * NT : 3 * NT],
                              hb[0:P, 2 * NT : 3 * NT])
            for k in range(1, KT):
                pick().dma_start(
                    head_sb[:, k * 4 * NT : k * 4 * NT + 3 * NT],
                    hb[k * P : (k + 1) * P, :],
                )
            # bulk x chunks 1..7 as k-halves (512KB pieces -> the consuming
            # super-group can start ~2us earlier), then the phase-B w
            # slices as 1MB single DMAs.
            for c in range(1, CH):
                half = KT * NT // 2
                for h in range(2):
                    pick().dma_start(
                        x_sb[:, (c - 1) * KT * NT + h * half
                             : (c - 1) * KT * NT + (h + 1) * half],
                        xq[(c - 1) * P : c * P, h * half : (h + 1) * half],
                    )
            for n in range(NA, NN):
                pick().dma_start(
                    w_sb[:, (n - NA) * KT * NT : (n - NA + 1) * KT * NT],
                    wq[(n - NA) * P : (n - NA + 1) * P, :],
                )

            # bias replication runs on gpsimd (idle anyway), off the PE.
            nc.gpsimd.partition_broadcast(bias_sb[:], bias_row[:])

            # ---- PE warm-up: keep the HAM activity window gapless from
            # queue-up until the first head pieces land.
            wu = psum_pool.tile([P, NT], f32, tag="ps", bufs=8, name="wu")
            for _ in range(WARM):
                nc.tensor.matmul(
                    wu[:, 0:P], lhsT=ones_sb[:], rhs=ones_sb[:],
                    start=True, stop=True,
                )

            # ---- main schedule ----
            def w_ap(n, k):
                if n < NA:
                    base = k * 4 * NT + (0 if n == 0 else 2 * NT)
                    return head_sb[:, base : base + NT]
                base = ((n - NA) * KT + k) * NT
                return w_sb[:, base : base + NT]

            def x_ap(c, k, j):
                if c == 0:
                    base = k * 4 * NT + NT + j * P
                    return head_sb[:, base : base + P]
                base = ((c - 1) * KT + k) * NT + j * P
                return x_sb[:, base : base + P]

            def mm(ps, c, j, n, k):
                nc.tensor.matmul(
                    ps[:],
                    lhsT=x_ap(c, k, j),
                    rhs=w_ap(n, k),
                    start=(k == 0),
                    stop=(k == KT - 1),
                )

            def ps_tile():
                return psum_pool.tile([P, NT], f32, tag="ps", bufs=8, name="ps")

            gctr = [0]

            def store(dst_ap, src_ap):
                eng = ring[gctr[0] % 2]
                gctr[0] += 1
                eng.dma_start(dst_ap, src_ap)

            # Phase A: per chunk, super-groups of (tiles x n0..1), k-outer,
            # each x tile stationary across its NA consecutive matmuls.
            def sga(c, js, k0_n_outer=False):
                pss = {(j, n): ps_tile() for j in js for n in range(NA)}
                for k in range(KT):
                    if k == 0 and k0_n_outer:
                        for n in range(NA):
                            for j in js:
                                mm(pss[(j, n)], c, j, n, k)
                        continue
                    for j in js:
                        for n in range(NA):
                            mm(pss[(j, n)], c, j, n, k)
                for j in js:
                    ob = out_pool.tile([P, NA * NT], bf16, tag="oba", bufs=8,
                                       name="ob")
                    for n in range(NA):
                        nc.vector.tensor_add(
                            ob[:, n * NT : (n + 1) * NT],
                            pss[(j, n)][:],
                            bias_sb[:, n * NT : (n + 1) * NT],
                        )
                    t = c * JJ + j
                    store(y[t * P : (t + 1) * P, 0 : NA * NT], ob[:])

            sga(0, [0, 1, 2, 3], k0_n_outer=True)  # head: 8 banks, one trio per k-step
            for c in range(1, CH):
                sga(c, [0, 1])
                sga(c, [2, 3])

            # Phase B: per token tile, n2..n5 k-outer / n-inner (width-4
            # stationary reuse), everything SBUF-resident.
            def blk(t, small_stores=False):
                c, j = divmod(t, JJ)
                pss = [ps_tile() for _ in range(NB)]
                for k in range(KT):
                    for g in range(NB):
                        mm(pss[g], c, j, NA + g, k)
                ob = out_pool.tile([P, NB * NT], bf16, tag="obb", bufs=6,
                                   name="ob")
                for g in range(NB):
                    nc.vector.tensor_add(
                        ob[:, g * NT : (g + 1) * NT],
                        pss[g][:],
                        bias_sb[:, (NA + g) * NT : (NA + g + 1) * NT],
                    )
                    if small_stores:
                        store(
                            y[t * P : (t + 1) * P,
                              (NA + g) * NT : (NA + g + 1) * NT],
                            ob[:, g * NT : (g + 1) * NT],
                        )
                if not small_stores:
                    store(y[t * P : (t + 1) * P, NA * NT : N3], ob[:])

            def blk_final(t):
                # n-major so drains/stores overlap remaining matmuls; the
                # last group drains and stores in halves on both rings.
                c, j = divmod(t, JJ)
                for g in range(NB):
                    ps = ps_tile()
                    for k in range(KT):
                        mm(ps, c, j, NA + g, k)
                    n = NA + g
                    ob = out_pool.tile([P, NT], bf16, tag="obf", bufs=2,
                                       name="ob")
                    parts = 2 if g == NB - 1 else 1
                    h = NT // parts
                    for i in range(parts):
                        sl = slice(i * h, (i + 1) * h)
                        nc.vector.tensor_add(
                            ob[:, sl], ps[:, sl],
                            bias_sb[:, n * NT + i * h : n * NT + (i + 1) * h],
                        )
                        store(
                            y[t * P : (t + 1) * P,
                              n * NT + i * h : n * NT + (i + 1) * h],
                            ob[:, sl],
                        )

            for t in range(T // P - 1):
                blk(t, small_stores=(t >= T // P - 3))
            blk_final(T // P - 1)
    nc.compile()
    return nc


def _get_nc():
    if "v3" not in _NC_CACHE:
        _NC_CACHE["v3"] = _build()
    return _NC_CACHE["v3"]


def _prep_in_maps(inputs):
    import ml_dtypes

    bf16 = ml_dtypes.bfloat16

    x = np.asarray(inputs["x"], dtype=np.float32)
    weight = np.asarray(inputs["weight"], dtype=np.float32)
    bias = np.asarray(inputs["bias"], dtype=np.float32)
    aq = np.asarray(inputs["A_q_pool"], dtype=np.float32)
    bq = np.asarray(inputs["B_q_pool"], dtype=np.float32)
    av = np.asarray(inputs["A_v_pool"], dtype=np.float32)
    bv = np.asarray(inputs["B_v_pool"], dtype=np.float32)
    idx = np.asarray(inputs["idx"]).reshape(B, -1)[:, 0].astype(np.int64)

    wt64 = weight.T.astype(np.float64)  # [D, N3]
    biasr = np.ascontiguousarray(bias.reshape(1, N3).astype(np.float32))
    xts = x.transpose(0, 2, 1)  # [B, D, T] strided view

    in_maps = []
    for b in range(B):
        i = int(idx[b])
        weff = wt64.copy()
        weff[:, :D] += SCALE * (aq[i].astype(np.float64) @ bq[i].astype(np.float64))
        weff[:, N3 - D:] += SCALE * (av[i].astype(np.float64) @ bv[i].astype(np.float64))
        weffb = weff.astype(np.float32).astype(bf16)        # [D, N3]
        xtb = np.ascontiguousarray(xts[b]).astype(bf16)     # [D, T]
        w4 = weffb.reshape(KT, P, NN, NT)
        x4 = xtb.reshape(KT, P, CH, NT)
        # head blob: rows k*P+p, cols [w(0,k) | x(0,k) | w(1,k)]
        hbb = np.ascontiguousarray(
            np.concatenate([w4[:, :, 0], x4[:, :, 0], w4[:, :, 1]], axis=2)
            .reshape(KT * P, 3 * NT)
        )
        # bulk w n2..5: rows (n-NA)*P+p, cols k*NT+c
        wqb = np.ascontiguousarray(
            w4[:, :, NA:].transpose(2, 1, 0, 3).reshape(NB * P, KT * NT)
        )
        # bulk x c1..7: rows (c-1)*P+p, cols k*NT+cc
        xqb = np.ascontiguousarray(
            x4[:, :, 1:].transpose(2, 1, 0, 3).reshape((CH - 1) * P, KT * NT)
        )
        in_maps.append({
            "hb": hbb,
            "wq": wqb,
            "xq": xqb,
            "biasr": biasr,
        })
    return in_maps


def _run(in_maps, trace=False, **kwargs):
    from concourse.bass_utils import run_bass_kernel_spmd

    nc = _get_nc()
    return run_bass_kernel_spmd(
        nc, in_maps, core_ids=list(range(B)), trace=trace, **kwargs
    )


def kernel(**inputs):
    res = _run(_prep_in_maps(inputs), trace=False)
    return np.stack(
        [np.asarray(r["y"], dtype=np.float32) for r in res.results], axis=0
    )
